# revision 1
# baseline (speedup 1.0000x reference)
"""Trainium2 Bass kernel for nn_Decoder sparse-attention decode step.

Math (algebraically reduced from the reference):
    phi1 = output[prev_node] @ W1.T + b1                      # [HID]
    u    = (phi1 @ W2) / sqrt(DH)                             # [H]
    cst  = (phi1 @ b2) / sqrt(DH)                             # scalar
    s[n]    = u . output[n] + cst                             # [N]
    attn[n] = (10*adj[n]) * tanh(s[n])                        # exact 0 where adj==0
    w = softmax(attn); w *= (attn != 0); p = w.max(); sel = argmax(w)

Device work per core (N/8 rows): masked matvec + tanh + softmax stats
(local max, sum of exp, masked max + its first index). Host combines the
8 stat quadruples exactly (online-softmax rescaling) -> no collectives.

Row layout on a core: padded shard of R = 128*JT rows; partition p holds
rows [JT*p, JT*(p+1)) in its free dim, so global row = JT*p + j.
"""

from contextlib import ExitStack

import numpy as np

import concourse.bass as bass
import concourse.bacc as bacc
import concourse.tile as tile
from concourse import mybir

F32 = mybir.dt.float32

N = 200000
H = 128
HID = 512
DH = 512.0
CLIP = 10.0
NCORES = 8
SHARD = N // NCORES            # 25000
JT = 196                       # rows per partition (padded)
RPAD = 128 * JT                # 25088
JC = 28                        # row-columns per chunk
NCH = JT // JC                 # 7 chunks
GSPLIT = 0                     # j-cols per chunk whose mul runs on GpSimd
BIGJ = 1.0e6                   # index-select sentinel (exact f32 int range)
BIGR = 1.0e7
NEG = -1.0e30


def build_program(reps=1, gsplit=GSPLIT, mode="full"):
    """mode: 'full' | 'dmaonly' (only the x DMAs per rep) |
    'nodma' (DMA once, repeat compute on stale tiles)."""
    nc = bacc.Bacc(
        "TRN2", target_bir_lowering=False, debug=False, num_devices=NCORES
    )

    x_d = nc.dram_tensor("x", [128, JT, H], F32, kind="ExternalInput").ap()
    adj_d = nc.dram_tensor("adj10", [128, JT], F32, kind="ExternalInput").ap()
    urep_d = nc.dram_tensor("urep", [128, JC * H], F32, kind="ExternalInput").ap()
    crep_d = nc.dram_tensor("crep", [128, 1], F32, kind="ExternalInput").ap()
    ident_d = nc.dram_tensor("ident", [128, 128], F32, kind="ExternalInput").ap()
    ones_d = nc.dram_tensor("ones128", [128, 1], F32, kind="ExternalInput").ap()
    onesr_d = nc.dram_tensor("onesr", [1, 128], F32, kind="ExternalInput").ap()
    out_d = nc.dram_tensor("o", [1, 4], F32, kind="ExternalOutput").ap()

    with tile.TileContext(nc) as tc, ExitStack() as ctx:
        const = ctx.enter_context(tc.tile_pool(name="const", bufs=1))
        xp = ctx.enter_context(tc.tile_pool(name="xp", bufs=3))
        pp = ctx.enter_context(tc.tile_pool(name="pp", bufs=2))
        sm = ctx.enter_context(tc.tile_pool(name="sm", bufs=1))
        ps = ctx.enter_context(tc.tile_pool(name="ps", bufs=1, space="PSUM"))

        urep = const.tile([128, JC * H], F32)
        nc.sync.dma_start(urep, urep_d)
        adj10 = const.tile([128, JT], F32)
        nc.sync.dma_start(adj10, adj_d)
        crep = const.tile([128, 1], F32)
        nc.sync.dma_start(crep, crep_d)
        ident = const.tile([128, 128], F32)
        nc.sync.dma_start(ident, ident_d)
        ones128 = const.tile([128, 1], F32)
        nc.sync.dma_start(ones128, ones_d)
        onesr = const.tile([1, 128], F32)
        nc.sync.dma_start(onesr, onesr_d)

        # column index + BIGJ, one row per partition: BIGJ + j
        jota = const.tile([128, JT], F32)
        nc.gpsimd.iota(
            jota, pattern=[[1, JT]], base=int(BIGJ), channel_multiplier=0,
            allow_small_or_imprecise_dtypes=True,
        )
        # partition base row: JT * p
        pbase = const.tile([128, 1], F32)
        nc.gpsimd.iota(
            pbase, pattern=[[0, 1]], base=0, channel_multiplier=JT,
            allow_small_or_imprecise_dtypes=True,
        )

        xts_fixed = None
        if mode == "nodma":
            xn = ctx.enter_context(tc.tile_pool(name="xn", bufs=1))
            xts_fixed = []
            for ch in range(NCH):
                xt = xn.tile([128, JC, H], F32, tag=f"xtf{ch}")
                nc.sync.dma_start(xt, x_d[:, ch * JC:(ch + 1) * JC, :])
                xts_fixed.append(xt)

        for _rep in range(reps):
            if mode == "dmaonly":
                for ch in range(NCH):
                    xt = xp.tile([128, JC, H], F32, tag="xt")
                    nc.sync.dma_start(xt, x_d[:, ch * JC:(ch + 1) * JC, :])
                continue

            s_all = sm.tile([128, JT], F32, tag="s_all")

            for ch in range(NCH):
                if xts_fixed is not None:
                    xt = xts_fixed[ch]
                else:
                    xt = xp.tile([128, JC, H], F32, tag="xt")
                    nc.sync.dma_start(xt, x_d[:, ch * JC:(ch + 1) * JC, :])
                pt = pp.tile([128, JC, H], F32, tag="pt")
                if gsplit > 0:
                    nc.gpsimd.tensor_tensor(
                        pt[:, 0:gsplit, :].rearrange("p a b -> p (a b)"),
                        xt[:, 0:gsplit, :].rearrange("p a b -> p (a b)"),
                        urep[:, 0:gsplit * H],
                        op=mybir.AluOpType.mult,
                    )
                    nc.vector.tensor_tensor(
                        pt[:, gsplit:JC, :].rearrange("p a b -> p (a b)"),
                        xt[:, gsplit:JC, :].rearrange("p a b -> p (a b)"),
                        urep[:, gsplit * H:JC * H],
                        op=mybir.AluOpType.mult,
                    )
                else:
                    nc.vector.tensor_tensor(
                        pt.rearrange("p a b -> p (a b)"),
                        xt.rearrange("p a b -> p (a b)"),
                        urep,
                        op=mybir.AluOpType.mult,
                    )
                nc.vector.tensor_reduce(
                    s_all[:, ch * JC:(ch + 1) * JC],
                    pt,
                    axis=mybir.AxisListType.X,
                    op=mybir.AluOpType.add,
                )

            # attn = (10*adj) * tanh(s + cst)
            attn_raw = sm.tile([128, JT], F32, tag="attn_raw")
            nc.scalar.activation(
                attn_raw, s_all, mybir.ActivationFunctionType.Tanh,
                bias=crep[:, 0:1], scale=1.0,
            )
            attn = sm.tile([128, JT], F32, tag="attn")
            nc.vector.tensor_tensor(
                attn, attn_raw, adj10, op=mybir.AluOpType.mult
            )
            # pad rows (x=0, adj10=0) yield attn == 0 exactly: excluded from
            # the masked argmax; host subtracts their exp(0-m_l) from z_l.

            # local (per-partition) stats
            m_p = sm.tile([128, 1], F32, tag="m_p")
            nc.vector.tensor_reduce(
                m_p, attn, axis=mybir.AxisListType.X, op=mybir.AluOpType.max
            )
            mask0 = sm.tile([128, JT], F32, tag="mask0")
            nc.vector.tensor_scalar(
                mask0, attn, 0.0, None, op0=mybir.AluOpType.is_equal
            )
            masked = sm.tile([128, JT], F32, tag="masked")
            nc.vector.scalar_tensor_tensor(
                masked, mask0, NEG, attn,
                op0=mybir.AluOpType.mult, op1=mybir.AluOpType.add,
            )
            mnz_p = sm.tile([128, 1], F32, tag="mnz_p")
            nc.vector.tensor_reduce(
                mnz_p, masked, axis=mybir.AxisListType.X,
                op=mybir.AluOpType.max,
            )
            # first j achieving the per-partition masked max
            cmask = sm.tile([128, JT], F32, tag="cmask")
            nc.vector.tensor_scalar(
                cmask, masked, mnz_p[:, 0:1], None,
                op0=mybir.AluOpType.is_equal,
            )
            cand = sm.tile([128, JT], F32, tag="cand")
            nc.vector.scalar_tensor_tensor(
                cand, cmask, -BIGJ, jota,
                op0=mybir.AluOpType.mult, op1=mybir.AluOpType.add,
            )
            jmin_p = sm.tile([128, 1], F32, tag="jmin_p")
            nc.vector.tensor_reduce(
                jmin_p, cand, axis=mybir.AxisListType.X, op=mybir.AluOpType.min
            )
            row_p = sm.tile([128, 1], F32, tag="row_p")
            nc.vector.tensor_tensor(
                row_p, pbase, jmin_p, op=mybir.AluOpType.add
            )

            # cross-partition combine: transpose each [128,1] stat to [1,128]
            # (AP partition offsets must be aligned, so keep everything on p0)
            ps3 = ps.tile([1, 384], F32, tag="ps3")
            nc.tensor.transpose(ps3[0:1, 0:128], m_p, ident)
            nc.tensor.transpose(ps3[0:1, 128:256], mnz_p, ident)
            nc.tensor.transpose(ps3[0:1, 256:384], row_p, ident)
            stats_t = sm.tile([1, 384], F32, tag="stats_t")
            nc.vector.tensor_copy(stats_t, ps3)

            m_l = sm.tile([1, 1], F32, tag="m_l")
            nc.vector.tensor_reduce(
                m_l, stats_t[0:1, 0:128], axis=mybir.AxisListType.X,
                op=mybir.AluOpType.max,
            )
            mnz_l = sm.tile([1, 1], F32, tag="mnz_l")
            nc.vector.tensor_reduce(
                mnz_l, stats_t[0:1, 128:256], axis=mybir.AxisListType.X,
                op=mybir.AluOpType.max,
            )
            rmask = sm.tile([1, 128], F32, tag="rmask")
            nc.vector.tensor_scalar(
                rmask, stats_t[0:1, 128:256], mnz_l[0:1, 0:1], None,
                op0=mybir.AluOpType.is_equal,
            )
            rows_b = sm.tile([1, 128], F32, tag="rows_b")
            nc.vector.tensor_scalar(
                rows_b, stats_t[0:1, 256:384], BIGR, None,
                op0=mybir.AluOpType.add,
            )
            cand_r = sm.tile([1, 128], F32, tag="cand_r")
            nc.vector.scalar_tensor_tensor(
                cand_r, rmask, -BIGR, rows_b,
                op0=mybir.AluOpType.mult, op1=mybir.AluOpType.add,
            )
            idx_l = sm.tile([1, 1], F32, tag="idx_l")
            nc.vector.tensor_reduce(
                idx_l, cand_r, axis=mybir.AxisListType.X, op=mybir.AluOpType.min
            )

            # broadcast -m_l to all partitions: ones_r.T @ m_l -> [128,1]
            mb_ps = ps.tile([128, 1], F32, tag="mb_ps")
            nc.tensor.matmul(mb_ps, onesr, m_l)
            neg_m = sm.tile([128, 1], F32, tag="neg_m")
            nc.vector.tensor_scalar(
                neg_m, mb_ps, -1.0, None, op0=mybir.AluOpType.mult
            )

            # z = sum over all entries of exp(attn - m_l)
            e_t = sm.tile([128, JT], F32, tag="e_t")
            z_p = sm.tile([128, 1], F32, tag="z_p")
            nc.scalar.activation(
                e_t, attn, mybir.ActivationFunctionType.Exp,
                bias=neg_m[:, 0:1], scale=1.0, accum_out=z_p,
            )
            z_ps = ps.tile([1, 1], F32, tag="z_ps")
            nc.tensor.matmul(z_ps, ones128, z_p)

            if _rep == reps - 1:
                fin = sm.tile([1, 4], F32, tag="fin")
                nc.vector.tensor_copy(fin[0:1, 0:1], m_l)
                nc.vector.tensor_copy(fin[0:1, 1:2], z_ps)
                nc.vector.tensor_copy(fin[0:1, 2:3], mnz_l)
                nc.vector.tensor_copy(fin[0:1, 3:4], idx_l)
                nc.sync.dma_start(out_d, fin)

        if mode == "dmaonly":
            fin = sm.tile([1, 4], F32, tag="fin")
            nc.vector.memset(fin, 0.0)
            nc.sync.dma_start(out_d, fin)

    nc.compile()
    return nc


P2 = RPAD // 512               # 49 partitions in the V2 s-layout
F32R = mybir.dt.float32r


def build_program_v2(reps=1, tr_f32r=False, mm_f32r=True, mode="full",
                     dma_split=False, nb=28, xbufs=4, tbufs=6):
    """PE-based variant: PE-transpose 128x128 blocks into PSUM (fp32), copy
    back to SBUF (DVE/ACT alternating, rounding to f32r on the way), then one
    accumulating f32r matmul per 512-row chunk with a sliding-window weight
    (u in column g) so chunk g's scores land on psum partition g.
    Row r = 512*p + j in the [49, 512] layout.

    The BIR verifier requires every f32r-matmult input to be *produced* as
    f32r (rounded) by an engine op, so the psum->sbuf copy does the rounding
    and the weight window gets a one-time rounding copy. tr_f32r is ignored
    (transposes stay fp32 for this reason)."""
    nc = bacc.Bacc(
        "TRN2", target_bir_lowering=False, debug=False, num_devices=NCORES
    )

    x_d = nc.dram_tensor("x", [JT, 128, H], F32, kind="ExternalInput").ap()
    adj_d = nc.dram_tensor("adj10", [P2, 512], F32, kind="ExternalInput").ap()
    uwin_d = nc.dram_tensor("uwin", [128, 257], F32, kind="ExternalInput").ap()
    crep_d = nc.dram_tensor("crep", [128, 1], F32, kind="ExternalInput").ap()
    ident_d = nc.dram_tensor("ident", [128, 128], F32, kind="ExternalInput").ap()
    ones_d = nc.dram_tensor("ones128", [128, 1], F32, kind="ExternalInput").ap()
    onesr_d = nc.dram_tensor("onesr", [1, 128], F32, kind="ExternalInput").ap()
    out_d = nc.dram_tensor("o", [1, 4], F32, kind="ExternalOutput").ap()

    NB = nb                     # blocks per DMA chunk
    assert JT % NB == 0 and NB % 4 == 0
    NGC = NB // 4               # psum groups per DMA chunk
    NDMA = JT // NB             # DMA chunks
    NG = JT // 4                # 49 groups of 4 blocks = 512 rows

    MMDT = F32R if mm_f32r else F32

    with tile.TileContext(nc) as tc, ExitStack() as ctx:
        const = ctx.enter_context(tc.tile_pool(name="const", bufs=1))
        xp = ctx.enter_context(tc.tile_pool(name="xp", bufs=xbufs))
        tp = ctx.enter_context(tc.tile_pool(name="tp", bufs=4, space="PSUM"))
        tt = ctx.enter_context(tc.tile_pool(name="tt", bufs=tbufs))
        sp = ctx.enter_context(tc.tile_pool(name="sp", bufs=1, space="PSUM"))
        sm = ctx.enter_context(tc.tile_pool(name="sm", bufs=1))
        ps = ctx.enter_context(tc.tile_pool(name="ps", bufs=1, space="PSUM"))

        uwin = const.tile([128, 257], F32)
        nc.sync.dma_start(uwin, uwin_d)
        uwin_r = const.tile([128, 257], MMDT)
        nc.vector.tensor_copy(uwin_r, uwin)
        adj10 = const.tile([P2, 512], F32)
        nc.sync.dma_start(adj10, adj_d)
        crep = const.tile([128, 1], F32)
        nc.sync.dma_start(crep, crep_d)
        ident = const.tile([128, 128], F32)
        nc.sync.dma_start(ident, ident_d)
        ones128 = const.tile([128, 1], F32)
        nc.sync.dma_start(ones128, ones_d)
        onesr = const.tile([1, 128], F32)
        nc.sync.dma_start(onesr, onesr_d)

        jota = const.tile([P2, 512], F32)
        nc.gpsimd.iota(
            jota, pattern=[[1, 512]], base=int(BIGJ), channel_multiplier=0,
            allow_small_or_imprecise_dtypes=True,
        )
        pbase = const.tile([P2, 1], F32)
        nc.gpsimd.iota(
            pbase, pattern=[[0, 1]], base=0, channel_multiplier=512,
            allow_small_or_imprecise_dtypes=True,
        )

        def chunk_dma(xt, c):
            src = x_d[c * NB:(c + 1) * NB, :, :].rearrange("b p h -> p b h")
            if dma_split:
                # alternate whole chunks between the two HWDGE queues
                eng = nc.sync if c % 2 == 0 else nc.scalar
                eng.dma_start(xt, src)
            else:
                nc.sync.dma_start(xt, src)

        xts_fixed = None
        if mode == "nodma":
            xn = ctx.enter_context(tc.tile_pool(name="xn", bufs=1))
            xts_fixed = []
            for c in range(NDMA):
                xt = xn.tile([128, NB, H], F32, tag=f"xtf{c}")
                chunk_dma(xt, c)
                xts_fixed.append(xt)

        for _rep in range(reps):
            if mode == "dmaonly":
                for c in range(NDMA):
                    xt = xp.tile([128, NB, H], F32, tag="xt")
                    chunk_dma(xt, c)
                continue

            s_acc = sp.tile([128, 512], F32, tag="s_acc")
            for c in range(NDMA):
                if xts_fixed is not None:
                    xt = xts_fixed[c]
                else:
                    xt = xp.tile([128, NB, H], F32, tag="xt")
                    chunk_dma(xt, c)
                for k in range(NGC):
                    g = c * NGC + k
                    pst = tp.tile([128, 512], F32, tag="pst")
                    for i in range(4):
                        nc.tensor.transpose(
                            pst[:, 128 * i:128 * (i + 1)],
                            xt[:, 4 * k + i, :],
                            ident,
                        )
                    xtt = tt.tile([128, 512], MMDT, tag="xtt")
                    if g % 2 == 0:
                        nc.vector.tensor_copy(xtt, pst)
                    else:
                        nc.scalar.copy(xtt, pst)
                    nc.tensor.matmul(
                        s_acc,
                        uwin_r[:, 128 - g:256 - g],
                        xtt,
                        start=(g == 0),
                        stop=(g == NG - 1),
                    )

            s_sb = sm.tile([P2, 512], F32, tag="s_sb")
            nc.vector.tensor_copy(s_sb, s_acc[0:P2, :])

            attn_raw = sm.tile([P2, 512], F32, tag="attn_raw")
            nc.scalar.activation(
                attn_raw, s_sb, mybir.ActivationFunctionType.Tanh,
                bias=crep[0:P2, 0:1], scale=1.0,
            )
            attn = sm.tile([P2, 512], F32, tag="attn")
            nc.vector.tensor_tensor(
                attn, attn_raw, adj10, op=mybir.AluOpType.mult
            )

            m_p = sm.tile([P2, 1], F32, tag="m_p")
            nc.vector.tensor_reduce(
                m_p, attn, axis=mybir.AxisListType.X, op=mybir.AluOpType.max
            )
            mask0 = sm.tile([P2, 512], F32, tag="mask0")
            nc.vector.tensor_scalar(
                mask0, attn, 0.0, None, op0=mybir.AluOpType.is_equal
            )
            masked = sm.tile([P2, 512], F32, tag="masked")
            nc.vector.scalar_tensor_tensor(
                masked, mask0, NEG, attn,
                op0=mybir.AluOpType.mult, op1=mybir.AluOpType.add,
            )
            mnz_p = sm.tile([P2, 1], F32, tag="mnz_p")
            nc.vector.tensor_reduce(
                mnz_p, masked, axis=mybir.AxisListType.X,
                op=mybir.AluOpType.max,
            )
            cmask = sm.tile([P2, 512], F32, tag="cmask")
            nc.vector.tensor_scalar(
                cmask, masked, mnz_p[:, 0:1], None,
                op0=mybir.AluOpType.is_equal,
            )
            cand = sm.tile([P2, 512], F32, tag="cand")
            nc.vector.scalar_tensor_tensor(
                cand, cmask, -BIGJ, jota,
                op0=mybir.AluOpType.mult, op1=mybir.AluOpType.add,
            )
            jmin_p = sm.tile([P2, 1], F32, tag="jmin_p")
            nc.vector.tensor_reduce(
                jmin_p, cand, axis=mybir.AxisListType.X, op=mybir.AluOpType.min
            )
            row_p = sm.tile([P2, 1], F32, tag="row_p")
            nc.vector.tensor_tensor(
                row_p, pbase, jmin_p, op=mybir.AluOpType.add
            )

            ps3 = ps.tile([1, 3 * P2], F32, tag="ps3")
            nc.tensor.transpose(ps3[0:1, 0:P2], m_p, ident[0:P2, 0:P2])
            nc.tensor.transpose(ps3[0:1, P2:2 * P2], mnz_p, ident[0:P2, 0:P2])
            nc.tensor.transpose(
                ps3[0:1, 2 * P2:3 * P2], row_p, ident[0:P2, 0:P2]
            )
            stats_t = sm.tile([1, 3 * P2], F32, tag="stats_t")
            nc.vector.tensor_copy(stats_t, ps3)

            m_l = sm.tile([1, 1], F32, tag="m_l")
            nc.vector.tensor_reduce(
                m_l, stats_t[0:1, 0:P2], axis=mybir.AxisListType.X,
                op=mybir.AluOpType.max,
            )
            mnz_l = sm.tile([1, 1], F32, tag="mnz_l")
            nc.vector.tensor_reduce(
                mnz_l, stats_t[0:1, P2:2 * P2], axis=mybir.AxisListType.X,
                op=mybir.AluOpType.max,
            )
            rmask = sm.tile([1, P2], F32, tag="rmask")
            nc.vector.tensor_scalar(
                rmask, stats_t[0:1, P2:2 * P2], mnz_l[0:1, 0:1], None,
                op0=mybir.AluOpType.is_equal,
            )
            rows_b = sm.tile([1, P2], F32, tag="rows_b")
            nc.vector.tensor_scalar(
                rows_b, stats_t[0:1, 2 * P2:3 * P2], BIGR, None,
                op0=mybir.AluOpType.add,
            )
            cand_r = sm.tile([1, P2], F32, tag="cand_r")
            nc.vector.scalar_tensor_tensor(
                cand_r, rmask, -BIGR, rows_b,
                op0=mybir.AluOpType.mult, op1=mybir.AluOpType.add,
            )
            idx_l = sm.tile([1, 1], F32, tag="idx_l")
            nc.vector.tensor_reduce(
                idx_l, cand_r, axis=mybir.AxisListType.X, op=mybir.AluOpType.min
            )

            mb_ps = ps.tile([P2, 1], F32, tag="mb_ps")
            nc.tensor.matmul(mb_ps, onesr[0:1, 0:P2], m_l)
            neg_m = sm.tile([P2, 1], F32, tag="neg_m")
            nc.vector.tensor_scalar(
                neg_m, mb_ps, -1.0, None, op0=mybir.AluOpType.mult
            )

            e_t = sm.tile([P2, 512], F32, tag="e_t")
            z_p = sm.tile([P2, 1], F32, tag="z_p")
            nc.scalar.activation(
                e_t, attn, mybir.ActivationFunctionType.Exp,
                bias=neg_m[:, 0:1], scale=1.0, accum_out=z_p,
            )
            z_ps = ps.tile([1, 1], F32, tag="z_ps")
            nc.tensor.matmul(z_ps, ones128[0:P2, 0:1], z_p)

            if _rep == reps - 1:
                fin = sm.tile([1, 4], F32, tag="fin")
                nc.vector.tensor_copy(fin[0:1, 0:1], m_l)
                nc.vector.tensor_copy(fin[0:1, 1:2], z_ps)
                nc.vector.tensor_copy(fin[0:1, 2:3], mnz_l)
                nc.vector.tensor_copy(fin[0:1, 3:4], idx_l)
                nc.sync.dma_start(out_d, fin)

        if mode == "dmaonly":
            fin = sm.tile([1, 4], F32, tag="fin")
            nc.vector.memset(fin, 0.0)
            nc.sync.dma_start(out_d, fin)

    nc.compile()
    return nc


def make_in_maps_v2(output, adj_modified, W1, b1, W2, b2, prev_node):
    maps = make_in_maps(output, adj_modified, W1, b1, W2, b2, prev_node)
    u = maps[0]["urep"][0, :H].copy()
    uwin = np.zeros((128, 257), dtype=np.float32)
    uwin[:, 128] = u
    out = []
    for m in maps:
        out.append({
            "x": m["x"].reshape(JT, 128, H),
            "adj10": m["adj10"].reshape(P2, 512),
            "uwin": uwin,
            "crep": m["crep"],
            "ident": m["ident"],
            "ones128": m["ones128"],
            "onesr": m["onesr"],
        })
    return out


_CACHE = {}


def _get_program():
    if "nc" not in _CACHE:
        _CACHE["nc"] = build_program_v2()
    return _CACHE["nc"]


def make_in_maps(output, adj_modified, W1, b1, W2, b2, prev_node):
    output = np.ascontiguousarray(np.asarray(output, dtype=np.float32))
    adj = np.asarray(adj_modified, dtype=np.float32)
    W1 = np.asarray(W1, dtype=np.float64)
    b1 = np.asarray(b1, dtype=np.float64)
    W2 = np.asarray(W2, dtype=np.float64)
    b2 = np.asarray(b2, dtype=np.float64)
    pn = int(np.asarray(prev_node))

    v_i = output[pn].astype(np.float64)
    phi1 = W1 @ v_i + b1                       # [HID]
    u = (phi1 @ W2) / np.sqrt(DH)              # [H]
    cst = float(phi1 @ b2) / np.sqrt(DH)

    urep = np.tile(u.astype(np.float32)[None, :], (128, JC)).astype(np.float32)
    crep = np.full((128, 1), np.float32(cst), dtype=np.float32)
    ident = np.eye(128, dtype=np.float32)
    ones128 = np.ones((128, 1), dtype=np.float32)
    onesr = np.ones((1, 128), dtype=np.float32)

    in_maps = []
    for c in range(NCORES):
        xs = np.zeros((RPAD, H), dtype=np.float32)
        xs[:SHARD] = output[c * SHARD:(c + 1) * SHARD]
        adjs = np.zeros((RPAD,), dtype=np.float32)
        adjs[:SHARD] = adj[c * SHARD:(c + 1) * SHARD] * np.float32(CLIP)
        in_maps.append({
            "x": xs.reshape(128, JT, H),
            "adj10": adjs.reshape(128, JT),
            "urep": urep,
            "crep": crep,
            "ident": ident,
            "ones128": ones128,
            "onesr": onesr,
        })
    return in_maps


def combine_stats(stats):
    """stats: [NCORES, 4] f32 rows of (m_l, z_l, mnz_l, idx_l)."""
    stats = np.asarray(stats, dtype=np.float64)
    m = stats[:, 0]
    z = stats[:, 1] - (RPAD - SHARD) * np.exp(0.0 - stats[:, 0])
    mnz = stats[:, 2]
    idx = stats[:, 3]
    m_g = m.max()
    z_g = float(np.sum(z * np.exp(m - m_g)))
    mnz_g = mnz.max()
    if mnz_g <= -1.0e29:
        return np.int32(0), np.float32(0.0)
    sel = min(
        int(round(idx[c])) + SHARD * c
        for c in range(NCORES)
        if mnz[c] == mnz_g
    )
    p = np.exp(mnz_g - m_g) / z_g
    return np.int32(sel), np.float32(p)


def kernel(output, adj_modified, W1, b1, W2, b2, prev_node):
    from concourse.bass_utils import run_bass_kernel_spmd

    nc = _get_program()
    in_maps = make_in_maps_v2(output, adj_modified, W1, b1, W2, b2, prev_node)
    res = run_bass_kernel_spmd(nc, in_maps, core_ids=list(range(NCORES)))
    stats = np.stack([res.results[c]["o"][0] for c in range(NCORES)])
    sel, p = combine_stats(stats)
    return sel, p



# revision 2
# speedup vs baseline: 98.3226x; 98.3226x over previous
"""Trainium2 Bass kernel for nn_Decoder sparse-attention decode step.

Math (algebraically reduced from the reference):
    phi1 = output[prev_node] @ W1.T + b1                      # [HID]
    u    = (phi1 @ W2) / sqrt(DH)                             # [H]
    cst  = (phi1 @ b2) / sqrt(DH)                             # scalar
    s[n]    = u . (adj[n] * output[n]) + cst                  # [N]
    attn[n] = 10 * tanh(s[n]) * adj[n]
    w = softmax(attn); w *= (attn != 0); p = w.max(); sel = argmax(w)

Since adj is binary, nodes with adj==0 have attn == 0 exactly and are
handled entirely on the host (count * exp(-m) in the softmax sum, never
the argmax winner because the argmax is taken over adj==1 nodes).  The
device therefore only sees the COMPACTED adj==1 rows (~N/2), packed on
the host, transposed to [H=128 partitions, M cols] and cast to fp16 —
halving HBM traffic twice over (compaction x dtype).

Device per core: G = M/512 accumulating fp16 matmuls with a sliding
one-hot weight window (u in column 128 of uwin) so group g's scores land
on psum partition g -> s [G, 512].  Tail: tanh(+cst bias), max, first
argmax index, exp(10*t - 10*max) with accumulated sum.  Host combines
the 8 (max, z, idx) triples exactly (online softmax) -> no collectives.

Pad columns are filled with -50*u/||u||^2 so s_pad ~= -50, tanh = -1:
never the max, and exp contributes ~e^-20 (ignored).
"""

from contextlib import ExitStack

import numpy as np

import concourse.bass as bass
import concourse.bacc as bacc
import concourse.tile as tile
from concourse import mybir

F32 = mybir.dt.float32
F16 = mybir.dt.float16

N = 200000
H = 128
HID = 512
DH = 512.0
CLIP = 10.0
NCORES = 8
SHARD = N // NCORES            # 25000
BIGJ = 1.0e6                   # index-select sentinel (exact f32 int range)
BIGR = 1.0e7


def _pick_cpg(G):
    for d in (5, 6, 4, 7, 3, 8, 2, 1):
        if G % d == 0:
            return d
    return 1


def build_program_v3(M, reps=1, mode="full", xbufs=3):
    """mode: 'full' | 'dmaonly' (only the x DMAs per rep) |
    'nodma' (DMA once, repeat compute on stale tiles)."""
    G = M // 512
    assert M % 512 == 0 and 1 <= G <= 128
    CPG = _pick_cpg(G)
    NDMA = G // CPG
    CH = CPG * 512

    nc = bacc.Bacc(
        "TRN2", target_bir_lowering=False, debug=False, num_devices=NCORES
    )

    xT_d = nc.dram_tensor("xt", [128, M], F16, kind="ExternalInput").ap()
    uwin_d = nc.dram_tensor("uwin", [128, 256], F16, kind="ExternalInput").ap()
    crep_d = nc.dram_tensor("crep", [128, 1], F32, kind="ExternalInput").ap()
    ident_d = nc.dram_tensor("ident", [128, 128], F32, kind="ExternalInput").ap()
    ones_d = nc.dram_tensor("ones128", [128, 1], F32, kind="ExternalInput").ap()
    onesr_d = nc.dram_tensor("onesr", [1, 128], F32, kind="ExternalInput").ap()
    out_d = nc.dram_tensor("o", [1, 4], F32, kind="ExternalOutput").ap()

    with tile.TileContext(nc) as tc, ExitStack() as ctx:
        const = ctx.enter_context(tc.tile_pool(name="const", bufs=1))
        xp = ctx.enter_context(tc.tile_pool(name="xp", bufs=xbufs))
        sp = ctx.enter_context(tc.tile_pool(name="sp", bufs=2, space="PSUM"))
        sm = ctx.enter_context(tc.tile_pool(name="sm", bufs=2))
        ps = ctx.enter_context(tc.tile_pool(name="ps", bufs=2, space="PSUM"))

        uwin = const.tile([128, 256], F16)
        nc.sync.dma_start(uwin, uwin_d)
        crep = const.tile([128, 1], F32)
        nc.sync.dma_start(crep, crep_d)
        ident = const.tile([128, 128], F32)
        nc.sync.dma_start(ident, ident_d)
        ones128 = const.tile([128, 1], F32)
        nc.sync.dma_start(ones128, ones_d)
        onesr = const.tile([1, 128], F32)
        nc.sync.dma_start(onesr, onesr_d)

        # column index + BIGJ per partition, and partition base 512*p
        jota = const.tile([128, 512], F32)
        nc.gpsimd.iota(
            jota, pattern=[[1, 512]], base=int(BIGJ), channel_multiplier=0,
            allow_small_or_imprecise_dtypes=True,
        )
        pbase = const.tile([128, 1], F32)
        nc.gpsimd.iota(
            pbase, pattern=[[0, 1]], base=0, channel_multiplier=512,
            allow_small_or_imprecise_dtypes=True,
        )

        xts_fixed = None
        if mode == "nodma":
            xn = ctx.enter_context(tc.tile_pool(name="xn", bufs=1))
            xts_fixed = []
            for c in range(NDMA):
                xt = xn.tile([128, CH], F16, tag=f"xtf{c}")
                nc.sync.dma_start(xt, xT_d[:, c * CH:(c + 1) * CH])
                xts_fixed.append(xt)

        for _rep in range(reps):
            if mode == "dmaonly":
                for c in range(NDMA):
                    xt = xp.tile([128, CH], F16, tag="xt")
                    nc.sync.dma_start(xt, xT_d[:, c * CH:(c + 1) * CH])
                continue

            s_acc = sp.tile([128, 512], F32, tag="s_acc")
            for c in range(NDMA):
                if xts_fixed is not None:
                    xt = xts_fixed[c]
                else:
                    xt = xp.tile([128, CH], F16, tag="xt")
                    nc.sync.dma_start(xt, xT_d[:, c * CH:(c + 1) * CH])
                for k in range(CPG):
                    g = c * CPG + k
                    nc.tensor.matmul(
                        s_acc,
                        uwin[:, 128 - g:256 - g],
                        xt[:, k * 512:(k + 1) * 512],
                        start=(g == 0),
                        stop=(g == G - 1),
                    )

            # t = tanh(s + cst); pads give exactly tanh(-50) == -1
            t_sb = sm.tile([G, 512], F32, tag="t_sb")
            nc.scalar.activation(
                t_sb, s_acc[0:G, :], mybir.ActivationFunctionType.Tanh,
                bias=crep[0:G, 0:1], scale=1.0,
            )

            # local max + first index achieving it
            m_p = sm.tile([G, 1], F32, tag="m_p")
            nc.vector.tensor_reduce(
                m_p, t_sb, axis=mybir.AxisListType.X, op=mybir.AluOpType.max
            )
            cmask = sm.tile([G, 512], F32, tag="cmask")
            nc.vector.tensor_scalar(
                cmask, t_sb, m_p[:, 0:1], None, op0=mybir.AluOpType.is_equal
            )
            cand = sm.tile([G, 512], F32, tag="cand")
            nc.vector.scalar_tensor_tensor(
                cand, cmask, -BIGJ, jota[0:G, :],
                op0=mybir.AluOpType.mult, op1=mybir.AluOpType.add,
            )
            jmin_p = sm.tile([G, 1], F32, tag="jmin_p")
            nc.vector.tensor_reduce(
                jmin_p, cand, axis=mybir.AxisListType.X, op=mybir.AluOpType.min
            )
            row_p = sm.tile([G, 1], F32, tag="row_p")
            nc.vector.tensor_tensor(
                row_p, pbase[0:G, :], jmin_p, op=mybir.AluOpType.add
            )

            # cross-partition combine (transpose stats to one partition)
            ps2 = ps.tile([1, 2 * G], F32, tag="ps2")
            nc.tensor.transpose(ps2[0:1, 0:G], m_p, ident[0:G, 0:G])
            nc.tensor.transpose(ps2[0:1, G:2 * G], row_p, ident[0:G, 0:G])
            stats_t = sm.tile([1, 2 * G], F32, tag="stats_t")
            nc.vector.tensor_copy(stats_t, ps2)

            m_l = sm.tile([1, 1], F32, tag="m_l")
            nc.vector.tensor_reduce(
                m_l, stats_t[0:1, 0:G], axis=mybir.AxisListType.X,
                op=mybir.AluOpType.max,
            )
            rmask = sm.tile([1, G], F32, tag="rmask")
            nc.vector.tensor_scalar(
                rmask, stats_t[0:1, 0:G], m_l[0:1, 0:1], None,
                op0=mybir.AluOpType.is_equal,
            )
            rows_b = sm.tile([1, G], F32, tag="rows_b")
            nc.vector.tensor_scalar(
                rows_b, stats_t[0:1, G:2 * G], BIGR, None,
                op0=mybir.AluOpType.add,
            )
            cand_r = sm.tile([1, G], F32, tag="cand_r")
            nc.vector.scalar_tensor_tensor(
                cand_r, rmask, -BIGR, rows_b,
                op0=mybir.AluOpType.mult, op1=mybir.AluOpType.add,
            )
            idx_l = sm.tile([1, 1], F32, tag="idx_l")
            nc.vector.tensor_reduce(
                idx_l, cand_r, axis=mybir.AxisListType.X, op=mybir.AluOpType.min
            )

            # broadcast -10*m_l to G partitions: onesr.T @ m_l then scale
            mb_ps = ps.tile([G, 1], F32, tag="mb_ps")
            nc.tensor.matmul(mb_ps, onesr[0:1, 0:G], m_l)
            neg_m = sm.tile([G, 1], F32, tag="neg_m")
            nc.vector.tensor_scalar(
                neg_m, mb_ps, -10.0, None, op0=mybir.AluOpType.mult
            )

            # z = sum exp(10*t - 10*m_l)
            e_t = sm.tile([G, 512], F32, tag="e_t")
            z_p = sm.tile([G, 1], F32, tag="z_p")
            nc.scalar.activation(
                e_t, t_sb, mybir.ActivationFunctionType.Exp,
                bias=neg_m[:, 0:1], scale=10.0, accum_out=z_p,
            )
            z_ps = ps.tile([1, 1], F32, tag="z_ps")
            nc.tensor.matmul(z_ps, ones128[0:G, 0:1], z_p)

            if _rep == reps - 1:
                fin = sm.tile([1, 4], F32, tag="fin")
                nc.vector.tensor_copy(fin[0:1, 0:1], m_l)
                nc.vector.tensor_copy(fin[0:1, 1:2], z_ps)
                nc.vector.tensor_copy(fin[0:1, 2:3], m_l)
                nc.vector.tensor_copy(fin[0:1, 3:4], idx_l)
                nc.sync.dma_start(out_d, fin)

        if mode == "dmaonly":
            fin = sm.tile([1, 4], F32, tag="fin")
            nc.vector.memset(fin, 0.0)
            nc.sync.dma_start(out_d, fin)

    nc.compile()
    return nc


def make_in_maps_v3(output, adj_modified, W1, b1, W2, b2, prev_node, M=None):
    """Returns (in_maps, idx_maps, nnz_list, M)."""
    output = np.ascontiguousarray(np.asarray(output, dtype=np.float32))
    adj = np.asarray(adj_modified, dtype=np.float32)
    W1 = np.asarray(W1, dtype=np.float64)
    b1 = np.asarray(b1, dtype=np.float64)
    W2 = np.asarray(W2, dtype=np.float64)
    b2 = np.asarray(b2, dtype=np.float64)
    pn = int(np.asarray(prev_node))

    v_i = output[pn].astype(np.float64)
    phi1 = W1 @ v_i + b1                       # [HID]
    u = (phi1 @ W2) / np.sqrt(DH)              # [H]
    cst = float(phi1 @ b2) / np.sqrt(DH)

    idx_maps, nnz_list = [], []
    for c in range(NCORES):
        idx = np.nonzero(adj[c * SHARD:(c + 1) * SHARD] != 0.0)[0]
        idx_maps.append(idx)
        nnz_list.append(len(idx))
    max_nnz = max(nnz_list)
    if M is None:
        M = 512 * ((max_nnz + 511) // 512)
        M = max(M, 512)
    assert max_nnz <= M

    u32 = u.astype(np.float32)
    uwin = np.zeros((128, 256), dtype=np.float16)
    uwin[:, 128] = u32.astype(np.float16)
    xpad = (-50.0 / float(u @ u)) * u
    xpad16 = xpad.astype(np.float16)
    crep = np.full((128, 1), np.float32(cst), dtype=np.float32)
    ident = np.eye(128, dtype=np.float32)
    ones128 = np.ones((128, 1), dtype=np.float32)
    onesr = np.ones((1, 128), dtype=np.float32)

    in_maps = []
    for c in range(NCORES):
        sh = output[c * SHARD:(c + 1) * SHARD]
        nnz = nnz_list[c]
        xT = np.empty((128, M), dtype=np.float16)
        xT[:, :nnz] = sh[idx_maps[c]].T.astype(np.float16)
        xT[:, nnz:] = xpad16[:, None]
        in_maps.append({
            "xt": xT,
            "uwin": uwin,
            "crep": crep,
            "ident": ident,
            "ones128": ones128,
            "onesr": onesr,
        })
    return in_maps, idx_maps, nnz_list, M


def combine_stats_v3(stats, idx_maps, nnz_list):
    """stats: [NCORES, 4] f32 rows of (m_t, z, m_t, idx)."""
    stats = np.asarray(stats, dtype=np.float64)
    m10 = 10.0 * stats[:, 0]
    z = stats[:, 1]
    idx = stats[:, 3]
    nnz_tot = int(sum(nnz_list))
    mnz_g = float(m10.max())
    m_g = max(mnz_g, 0.0)
    z_g = float(np.sum(z * np.exp(m10 - m_g)))
    if nnz_tot < N:
        z_g += (N - nnz_tot) * np.exp(0.0 - m_g)
    sel = min(
        int(idx_maps[c][min(int(round(idx[c])), nnz_list[c] - 1)]) + SHARD * c
        for c in range(NCORES)
        if m10[c] == mnz_g
    )
    p = np.exp(mnz_g - m_g) / z_g
    return np.int32(sel), np.float32(p)


_CACHE = {}


def _get_program(M):
    key = ("v3", M)
    if key not in _CACHE:
        _CACHE[key] = build_program_v3(M)
    return _CACHE[key]


def kernel(output, adj_modified, W1, b1, W2, b2, prev_node):
    from concourse.bass_utils import run_bass_kernel_spmd

    in_maps, idx_maps, nnz_list, M = make_in_maps_v3(
        output, adj_modified, W1, b1, W2, b2, prev_node
    )
    nc = _get_program(M)
    res = run_bass_kernel_spmd(nc, in_maps, core_ids=list(range(NCORES)))
    stats = np.stack([res.results[c]["o"][0] for c in range(NCORES)])
    sel, p = combine_stats_v3(stats, idx_maps, nnz_list)
    return sel, p
